# revision 26
# baseline (speedup 1.0000x reference)
"""Trainium2 Bass kernel for nn_AttnGate_5712306504201.

Pooled (mean||max over blocks of 16) GQA block-attention:
  qh = pool_cat(q) @ wq ; kh = pool_cat(k) @ wk   (per-head)
  RoPE(qh, kh) ; attn = softmax(mask(qh @ kh^T / sqrt(128)))

Shapes: B=2, HQ=32, HK=8, S=8192, D=128, HID=128, BS=16, NB=512.
Output: [2, 32, 512, 512] fp32.

Sharding (8 cores): core c -> batch c//4, q-head group g=c%4
(q heads 8g..8g+7, kv heads 2g..2g+1). Outputs are disjoint; no
collectives.

Per-core dataflow (fp16 device data, fp32 accumulation):
 - host pre-permutes seq to "j-major" order (pos = j*512 + blk) and
   pre-transposes to [head, d, seq] fp16
 - ALL constants are host-packed into one [128, CW] f16 tensor and
   loaded with a single DMA (small individual DMAs measured ~2.5us
   fixed latency each and serialized the queue for ~25us, starving
   the first head's load)
 - head loads alternate between the sync and scalar HWDGE queues in
   consumption order; the first two heads are split into quarter/half
   DMAs so pooling starts earlier; scalar-queue DMAs are issued before
   any Act compute (avoids head-of-line blocking of that DGE queue)
 - max-pool: halving tensor_max tree on DVE (2x packed mode; DVE is
   the only engine that can run TensorTensor — they are illegal on
   Pool/GpSimd, and scalar_tensor_tensor measured 1x)
 - mean-pool is folded into the projection: 16 accumulating PE matmuls
   over the 16 j-slabs + 1 matmul for the max features
 - RoPE in [hid, blk] layout; rotate_half runs as a PE matmul with a
   signed permutation matrix
 - attention per 128-row q-tile with causal N truncation; staircase
   bias PSUM-preloaded via identity matmul on the DIAGONAL block only
   (single stop=True on the last matmul of each PSUM group); attention
   is software-pipelined one q-head behind projection so the PE
   instruction stream stays dense (pstate ramp)
 - softmax: ScalarE Exp -> f16 into a per-head [128, 4, 512] SBUF
   buffer; ONE store DMA per q-head (32 small stores measured a ~25us
   serial tail); columns beyond the causal prefix hold stale garbage,
   masked out on the host during row normalization
"""

import os
import sys

import numpy as np

for _p in ("/opt/trn_rl_repo", "/root/.axon_site/_ro/trn_rl_repo"):
    if os.path.isdir(_p) and _p not in sys.path:
        sys.path.insert(0, _p)

B, HQ, HK, S, D, HID, BS = 2, 32, 8, 8192, 128, 128, 16
NB = S // BS  # 512
N_CORES = 8
QH_PER_CORE = HQ // 4
KH_PER_CORE = 2
QTILES = NB // 128  # 4
ATTN_SCALE = 1.0 / np.sqrt(np.float32(HID))

_PROGRAMS = {}


# NOTE: walrus's --enable-ldw-opt=true (LDWEIGHTS elision) crashes its
# codegen (visitInstLdweights assertion) — concourse hardcodes it off
# for a reason; the per-matmul LDWEIGHTS (~100ns) cannot be avoided.

# packed-constant column offsets (f16 columns of a [128, CW] tensor)
O_WK = 0                       # [n_kh, 2, 128] -> 512 cols
O_WQ = 512                     # [n_qh, 2, 128] -> 2048 cols
O_ROT = 2560                   # 128
O_IDENT = 2688                 # 128
O_COS = 2816                   # 512
O_SIN = 3328                   # 512
O_BIAS = 3840                  # 128 (causal) or QTILES*512 (dense)
CW_CAUSAL = 3968
CW_DENSE = 3840 + QTILES * NB


def _build_program(causal, n_qh=QH_PER_CORE, n_kh=KH_PER_CORE):
    """Build the per-core Bass program (SPMD, same program all cores)."""
    from contextlib import ExitStack

    import concourse.bass as bass
    import concourse.tile as tile
    from concourse import bacc, mybir

    f16 = mybir.dt.float16
    f32 = mybir.dt.float32
    FX = mybir.ActivationFunctionType

    nc = bacc.Bacc(
        "TRN2",
        target_bir_lowering=False,
        debug=False,
        enable_asserts=False,
        num_devices=N_CORES,
    )

    CW = CW_CAUSAL if causal else CW_DENSE
    q_d = nc.dram_tensor("q16", [n_qh, D, S], f16, kind="ExternalInput").ap()
    k_d = nc.dram_tensor("k16", [n_kh, D, S], f16, kind="ExternalInput").ap()
    cpack_d = nc.dram_tensor("cpack", [128, CW], f16, kind="ExternalInput").ap()
    # same memory layout as [n_qh, NB, NB]; the 4D shape lets the packed
    # per-head store express DRAM row t*128+p <- SBUF (p, t) as a plain
    # dimension permutation
    out_d = nc.dram_tensor(
        "attn_out", [n_qh, QTILES, 128, NB], f16, kind="ExternalOutput"
    ).ap()

    n_heads = n_kh + n_qh
    H = S // 2  # 4096
    Q = S // 4  # 2048

    with tile.TileContext(nc) as tc, ExitStack() as ctx:
        consts = ctx.enter_context(tc.tile_pool(name="consts", bufs=1))
        # 8 bufs: slots 0-7 get fresh ring buffers so no HWDGE load ever
        # waits on a tile-free (a 6-deep ring deadlocked: the Act-queue's
        # slot-7 load waited on a buffer freed only by compute that needed
        # Act to progress past that very DMA). Only the two SWDGE loads
        # (slots 8, 9) wait, on kv0/kv1 freeing at ~12-16us.
        raw_pool = ctx.enter_context(tc.tile_pool(name="raw", bufs=8))
        tree_pool = ctx.enter_context(tc.tile_pool(name="tree", bufs=2))
        head_pool = ctx.enter_context(tc.tile_pool(name="head", bufs=3))
        qhat_pool = ctx.enter_context(tc.tile_pool(name="qhat", bufs=3))
        ex_pool = ctx.enter_context(tc.tile_pool(name="ex", bufs=4))
        psum_proj = ctx.enter_context(tc.tile_pool(name="pproj", bufs=2, space="PSUM"))
        psum_rope = ctx.enter_context(tc.tile_pool(name="prope", bufs=2, space="PSUM"))
        psum_attn = ctx.enter_context(tc.tile_pool(name="pattn", bufs=4, space="PSUM"))

        # raw tiles allocated in CONSUMPTION order (ring-buffer slot k
        # pairs with slot k-6's free), DMAs issued in per-queue priority
        # order afterwards
        def head_src(slot):
            return (k_d, slot) if slot < n_kh else (q_d, slot - n_kh)

        raw_tiles = [
            raw_pool.tile([128, S], f16, tag="x", name=f"x{s}")
            for s in range(n_heads)
        ]

        # ---- SWDGE (gpsimd) takes the two LAST-consumed heads at t=0:
        # even at its slow ~100 GB/s they arrive long before needed, and
        # the HWDGE queues finish the other eight heads ~15us earlier ----
        src, idx = head_src(8)
        nc.gpsimd.dma_start(out=raw_tiles[8], in_=src[idx])
        src, idx = head_src(9)
        nc.gpsimd.dma_start(out=raw_tiles[9][:, 0:H], in_=src[idx, :, 0:H])
        nc.gpsimd.dma_start(out=raw_tiles[9][:, H:S], in_=src[idx, :, H:S])

        # ---- scalar-queue loads, first in Act program order ----
        x1 = raw_tiles[1]
        src, idx = head_src(1)
        nc.scalar.dma_start(out=x1[:, 0:H], in_=src[idx, :, 0:H])
        nc.scalar.dma_start(out=x1[:, H:S], in_=src[idx, :, H:S])
        for slot in (3, 5, 7):
            src, idx = head_src(slot)
            nc.scalar.dma_start(out=raw_tiles[slot], in_=src[idx])
        del x1

        # ---- sync queue: kv0's first quarters, then the packed consts,
        # then the rest (kv0 pooling starts ~4us in; the first projection
        # matmul only needs the consts at ~9us) ----
        x0 = raw_tiles[0]
        src, idx = head_src(0)
        for qq in range(2):
            nc.sync.dma_start(
                out=x0[:, qq * Q : (qq + 1) * Q], in_=src[idx, :, qq * Q : (qq + 1) * Q]
            )
        cpack = consts.tile([128, CW], f16)
        nc.sync.dma_start(out=cpack, in_=cpack_d)
        for qq in range(2, 4):
            nc.sync.dma_start(
                out=x0[:, qq * Q : (qq + 1) * Q], in_=src[idx, :, qq * Q : (qq + 1) * Q]
            )

        def wslab(is_q, head, chunk):
            base = O_WQ if is_q else O_WK
            o = base + (head * 2 + chunk) * HID
            return cpack[:, o : o + HID]

        rot_sb = cpack[:, O_ROT : O_ROT + HID]
        ident_sb = cpack[:, O_IDENT : O_IDENT + 128]
        cos_sb = cpack[:, O_COS : O_COS + NB]
        sin_sb = cpack[:, O_SIN : O_SIN + NB]

        def bias_sb(t):
            if causal:
                return cpack[:, O_BIAS : O_BIAS + 128]
            return cpack[:, O_BIAS + t * NB : O_BIAS + (t + 1) * NB]

        shift_sb = consts.tile([128, 1], f32)
        nc.vector.memset(shift_sb, -3.0)
        khat_all = consts.tile([HID, n_kh, NB], f16)

        for slot in (2, 4, 6):
            src, idx = head_src(slot)
            nc.sync.dma_start(out=raw_tiles[slot], in_=src[idx])

        def max_tree(slot):
            """Halving max tree over the 16 j-slabs -> tr[:, 0:NB]."""
            x = raw_tiles[slot]
            tr = tree_pool.tile([128, H], f16, tag="tr")
            if slot == 0:
                for qq in range(4):
                    o = qq * Q // 2
                    nc.vector.tensor_max(
                        tr[:, o : o + Q // 2],
                        x[:, qq * Q : qq * Q + Q // 2],
                        x[:, qq * Q + Q // 2 : (qq + 1) * Q],
                    )
                    nc.vector.tensor_max(
                        tr[:, o : o + NB], tr[:, o : o + NB], tr[:, o + NB : o + Q // 2]
                    )
                nc.vector.tensor_max(tr[:, 0:NB], tr[:, 0:NB], tr[:, 1024:1536])
                nc.vector.tensor_max(tr[:, 2048:2560], tr[:, 2048:2560], tr[:, 3072:3584])
                nc.vector.tensor_max(tr[:, 0:NB], tr[:, 0:NB], tr[:, 2048:2560])
            elif slot in (1, 9):
                for hh in range(2):
                    o = hh * H // 2
                    nc.vector.tensor_max(
                        tr[:, o : o + H // 2],
                        x[:, hh * H : hh * H + H // 2],
                        x[:, hh * H + H // 2 : (hh + 1) * H],
                    )
                    nc.vector.tensor_max(
                        tr[:, o : o + Q // 2], tr[:, o : o + Q // 2], tr[:, o + Q // 2 : o + Q]
                    )
                    nc.vector.tensor_max(
                        tr[:, o : o + NB], tr[:, o : o + NB], tr[:, o + NB : o + Q // 2]
                    )
                nc.vector.tensor_max(tr[:, 0:NB], tr[:, 0:NB], tr[:, 2048:2560])
            else:
                nc.vector.tensor_max(tr, x[:, 0:H], x[:, H:S])
                nc.vector.tensor_max(tr[:, 0 : H // 2], tr[:, 0 : H // 2], tr[:, H // 2 : H])
                nc.vector.tensor_max(tr[:, 0 : H // 4], tr[:, 0 : H // 4], tr[:, H // 4 : H // 2])
                nc.vector.tensor_max(tr[:, 0:NB], tr[:, 0:NB], tr[:, NB : 2 * NB])
            return tr

        def pool_project_rope(slot, is_q, w_head_idx, dst_ap):
            x = raw_tiles[slot]
            tr = max_tree(slot)
            mx = tr[:, 0:NB]

            ph = psum_proj.tile([HID, NB], f32, tag="proj")
            # (a stride-0 PSUM-output matmul that fuses the 16 j-slab
            # accumulations into one instruction fails the walrus ISA
            # encoding check — keep the 16-matmul chain; the redundant
            # LDWEIGHTS between them are elided by --enable-ldw-opt)
            for j in range(16):
                nc.tensor.matmul(
                    ph,
                    lhsT=wslab(is_q, w_head_idx, 0),
                    rhs=x[:, j * NB : (j + 1) * NB],
                    start=(j == 0),
                    stop=False,
                )
            nc.tensor.matmul(
                ph, lhsT=wslab(is_q, w_head_idx, 1), rhs=mx, start=False, stop=True
            )

            # h and R@h land side by side in one [128, 2*NB] tile so the
            # cos/sin multiply is ONE DVE op against the (adjacent in
            # cpack) cos|sin block
            hr = head_pool.tile([HID, 2 * NB], f16, tag="hr")
            nc.scalar.copy(hr[:, 0:NB], ph)
            rps = psum_rope.tile([HID, NB], f32, tag="rps")
            nc.tensor.matmul(rps, lhsT=rot_sb, rhs=hr[:, 0:NB], start=True, stop=True)
            nc.scalar.copy(hr[:, NB : 2 * NB], rps)
            ab = head_pool.tile([HID, 2 * NB], f16, tag="ab")
            nc.vector.tensor_mul(ab, hr, cpack[:, O_COS : O_COS + 2 * NB])
            nc.vector.tensor_add(dst_ap, ab[:, 0:NB], ab[:, NB : 2 * NB])

        store_n = [0]

        def attn_block(i, qhat):
            kv = min(i // 4, n_kh - 1)
            ex = ex_pool.tile([128, QTILES, NB], f16, tag="ex", name=f"ex{i}")
            for t in range(QTILES):
                ni = 128 * (t + 1) if causal else NB
                att = psum_attn.tile([128, NB], f32, tag="att")
                qh_t = qhat[:, t * 128 : (t + 1) * 128]
                if causal:
                    nc.tensor.matmul(
                        att[:, ni - 128 : ni], lhsT=ident_sb, rhs=bias_sb(t),
                        start=True, stop=False,
                    )
                    nc.tensor.matmul(
                        att[:, ni - 128 : ni],
                        lhsT=qh_t,
                        rhs=khat_all[:, kv, ni - 128 : ni],
                        start=False, stop=(ni == 128),
                    )
                    if ni > 128:
                        nc.tensor.matmul(
                            att[:, 0 : ni - 128],
                            lhsT=qh_t,
                            rhs=khat_all[:, kv, 0 : ni - 128],
                            start=True, stop=True,
                        )
                else:
                    nc.tensor.matmul(
                        att[:, 0:ni], lhsT=ident_sb, rhs=bias_sb(t),
                        start=True, stop=False,
                    )
                    nc.tensor.matmul(
                        att[:, 0:ni], lhsT=qh_t, rhs=khat_all[:, kv, 0:ni],
                        start=False, stop=True,
                    )

                nc.scalar.activation(
                    ex[:, t, 0:ni], att[:, 0:ni], FX.Exp, bias=shift_sb, scale=1.0
                )
            # one packed store per q-head: DRAM row t*128+p <- SBUF (p, t)
            eng = nc.sync if store_n[0] % 2 == 0 else nc.scalar
            store_n[0] += 1
            eng.dma_start(out=out_d[i].rearrange("t p c -> p t c"), in_=ex)

        for kv in range(n_kh):
            pool_project_rope(kv, False, kv, khat_all[:, kv, :])

        qhats = [None] * n_qh
        for i in range(n_qh):
            qhat = qhat_pool.tile([HID, NB], f16, tag="qhat", name=f"qhat{i}")
            qhats[i] = qhat
            pool_project_rope(n_kh + i, True, i, qhat)
            if i >= 1:
                attn_block(i - 1, qhats[i - 1])
        attn_block(n_qh - 1, qhats[n_qh - 1])

    nc.compile()
    return nc


def _get_program(causal):
    key = (causal, QH_PER_CORE, KH_PER_CORE)
    if key not in _PROGRAMS:
        _PROGRAMS[key] = _build_program(causal)
    return _PROGRAMS[key]


def _rot_matrix():
    r = np.zeros((HID, HID), dtype=np.float16)
    for d in range(64):
        r[d, 64 + d] = -1.0
        r[64 + d, d] = 1.0
    return np.ascontiguousarray(r.T)


def _jmajor_f16(x):
    """[h, S, D] fp32 -> transposed [h, D, S] fp16 with j-major seq order
    (seq index j*NB + blk for original position blk*BS + j)."""
    h = x.shape[0]
    xt = x.reshape(h, NB, BS, D).transpose(0, 3, 2, 1)
    return np.ascontiguousarray(xt.reshape(h, D, S).astype(np.float16))


def _prep(q, k, attention_mask, cos, sin, wq, wk):
    q = np.asarray(q, dtype=np.float32)
    k = np.asarray(k, dtype=np.float32)
    mask = np.asarray(attention_mask).astype(bool)
    cos = np.asarray(cos, dtype=np.float32)
    sin = np.asarray(sin, dtype=np.float32)
    wq = np.asarray(wq, dtype=np.float32)
    wk = np.asarray(wk, dtype=np.float32)

    tril = np.tril(np.ones((NB, NB), dtype=bool))
    causal = all(np.array_equal(mask[b, 0], tril) for b in range(B))

    wq_m = wq[:, :D, :] * (ATTN_SCALE / BS)
    wq_x = wq[:, D:, :] * ATTN_SCALE
    wk_m = wk[:, :D, :] / BS
    wk_x = wk[:, D:, :]
    # [128(d), head, chunk, hid]
    wqT = np.stack([wq_m, wq_x], axis=1).transpose(2, 0, 1, 3).astype(np.float16)
    wkT = np.stack([wk_m, wk_x], axis=1).transpose(2, 0, 1, 3).astype(np.float16)

    cosT = cos.transpose(0, 2, 1).astype(np.float16)  # [B, 128, 512]
    sinT = sin.transpose(0, 2, 1).astype(np.float16)
    rotT = _rot_matrix()
    ident128 = np.eye(128, dtype=np.float16)
    if causal:
        biasB = [
            np.where(np.tril(np.ones((128, 128), dtype=bool)), 0.0, -60000.0).astype(
                np.float16
            )
        ] * B
    else:
        nb = np.where(mask[:, 0], 0.0, -60000.0).astype(np.float16)
        biasB = [
            np.concatenate([nb[b].reshape(QTILES, 128, NB)[t] for t in range(QTILES)], axis=1)
            for b in range(B)
        ]

    in_maps = []
    for c in range(N_CORES):
        b, g = c // 4, c % 4
        qs = _jmajor_f16(q[b, 8 * g : 8 * g + 8])
        ks = _jmajor_f16(k[b, 2 * g : 2 * g + 2])
        cp = np.concatenate(
            [
                wkT[:, 2 * g : 2 * g + 2].reshape(128, -1),
                wqT[:, 8 * g : 8 * g + 8].reshape(128, -1),
                rotT,
                ident128,
                cosT[b],
                sinT[b],
                biasB[b],
            ],
            axis=1,
        )
        m = {"q16": qs, "k16": ks, "cpack": np.ascontiguousarray(cp)}
        in_maps.append(m)
    return causal, in_maps


def _postprocess(results, causal):
    out = np.zeros((B, HQ, NB, NB), dtype=np.float32)
    tril = np.tril(np.ones((NB, NB), dtype=np.float32)) if causal else None
    for c in range(N_CORES):
        b, g = c // 4, c % 4
        ex = results[c]["attn_out"].reshape(QH_PER_CORE, NB, NB).astype(np.float32)
        if causal:
            # columns beyond the causal prefix hold stale device garbage
            ex = np.where(tril > 0, ex, 0.0)
        sums = ex.sum(axis=-1, keepdims=True)
        out[b, 8 * g : 8 * g + 8] = np.where(
            sums > 0, ex / np.maximum(sums, 1e-30), np.float32(1.0 / NB)
        )
    return out


def kernel(q, k, attention_mask, cos, sin, wq, wk):
    from concourse import bass_utils

    causal, in_maps = _prep(q, k, attention_mask, cos, sin, wq, wk)
    nc = _get_program(causal)
    res = bass_utils.run_bass_kernel_spmd(nc, in_maps, core_ids=list(range(N_CORES)))
    return _postprocess(res.results, causal)


# revision 31
# speedup vs baseline: 1.1429x; 1.1429x over previous
"""Trainium2 Bass kernel for nn_AttnGate_5712306504201.

Pooled (mean||max over blocks of 16) GQA block-attention:
  qh = pool_cat(q) @ wq ; kh = pool_cat(k) @ wk   (per-head)
  RoPE(qh, kh) ; attn = softmax(mask(qh @ kh^T / sqrt(128)))

Shapes: B=2, HQ=32, HK=8, S=8192, D=128, HID=128, BS=16, NB=512.
Output: [2, 32, 512, 512] fp32.

Sharding (8 cores): core c -> batch c//4, q-head group g=c%4
(q heads 8g..8g+7, kv heads 2g..2g+1). Outputs are disjoint; no
collectives.

Per-core dataflow (fp16 device data, fp32 accumulation):
 - host pre-permutes seq to "j-major" order (pos = j*512 + blk) and
   pre-transposes to [head, d, seq] fp16
 - ALL constants are host-packed into one [128, CW] f16 tensor and
   loaded with a single DMA (small individual DMAs measured ~2.5us
   fixed latency each and serialized the queue for ~25us, starving
   the first head's load)
 - head loads alternate between the sync and scalar HWDGE queues in
   consumption order; the first two heads are split into quarter/half
   DMAs so pooling starts earlier; scalar-queue DMAs are issued before
   any Act compute (avoids head-of-line blocking of that DGE queue)
 - max-pool: halving tensor_max tree on DVE (2x packed mode; DVE is
   the only engine that can run TensorTensor — they are illegal on
   Pool/GpSimd, and scalar_tensor_tensor measured 1x)
 - mean-pool is folded into the projection: 16 accumulating PE matmuls
   over the 16 j-slabs + 1 matmul for the max features
 - RoPE in [hid, blk] layout; rotate_half runs as a PE matmul with a
   signed permutation matrix
 - attention per 128-row q-tile with causal N truncation; staircase
   bias PSUM-preloaded via identity matmul on the DIAGONAL block only
   (single stop=True on the last matmul of each PSUM group); attention
   is software-pipelined one q-head behind projection so the PE
   instruction stream stays dense (pstate ramp)
 - softmax: ScalarE Exp -> f16 into a per-head [128, 4, 512] SBUF
   buffer; ONE store DMA per q-head (32 small stores measured a ~25us
   serial tail); columns beyond the causal prefix hold stale garbage,
   masked out on the host during row normalization
"""

import os
import sys

import numpy as np

for _p in ("/opt/trn_rl_repo", "/root/.axon_site/_ro/trn_rl_repo"):
    if os.path.isdir(_p) and _p not in sys.path:
        sys.path.insert(0, _p)

B, HQ, HK, S, D, HID, BS = 2, 32, 8, 8192, 128, 128, 16
NB = S // BS  # 512
N_CORES = 8
QH_PER_CORE = HQ // 4
KH_PER_CORE = 2
QTILES = NB // 128  # 4
ATTN_SCALE = 1.0 / np.sqrt(np.float32(HID))

_PROGRAMS = {}


# NOTE: walrus's --enable-ldw-opt=true (LDWEIGHTS elision) crashes its
# codegen (visitInstLdweights assertion) — concourse hardcodes it off
# for a reason; the per-matmul LDWEIGHTS (~100ns) cannot be avoided.

# packed-constant column offsets (f16 columns of a [128, CW] tensor)
O_WK = 0                       # [n_kh, 2, 128] -> 512 cols
O_WQ = 512                     # [n_qh, 2, 128] -> 2048 cols
O_ROT = 2560                   # 128
O_IDENT = 2688                 # 128
O_COS = 2816                   # 512
O_SIN = 3328                   # 512
O_BIAS = 3840                  # 128 (causal) or QTILES*512 (dense)
CW_CAUSAL = 3968
CW_DENSE = 3840 + QTILES * NB


def _build_program(causal, n_qh=QH_PER_CORE, n_kh=KH_PER_CORE):
    """Build the per-core Bass program (SPMD, same program all cores)."""
    from contextlib import ExitStack

    import concourse.bass as bass
    import concourse.tile as tile
    from concourse import bacc, mybir

    f16 = mybir.dt.float16
    f32 = mybir.dt.float32
    FX = mybir.ActivationFunctionType

    nc = bacc.Bacc(
        "TRN2",
        target_bir_lowering=False,
        debug=False,
        enable_asserts=False,
        num_devices=N_CORES,
    )

    CW = CW_CAUSAL if causal else CW_DENSE
    q_d = nc.dram_tensor("q16", [n_qh, D, S], f16, kind="ExternalInput").ap()
    k_d = nc.dram_tensor("k16", [n_kh, D, S], f16, kind="ExternalInput").ap()
    cpack_d = nc.dram_tensor("cpack", [128, CW], f16, kind="ExternalInput").ap()
    # PARTITION-MAJOR output layout: DRAM (p, t, c) matches the SBUF
    # per-head exp buffer exactly, so each store writes 4KB-contiguous
    # runs per partition (the row-major [NB, NB] layout produced 1KB
    # scattered writes that crawled at ~105 GB/s). The host un-permutes.
    out_d = nc.dram_tensor(
        "attn_out", [n_qh, 128, QTILES, NB], f16, kind="ExternalOutput"
    ).ap()

    n_heads = n_kh + n_qh
    H = S // 2  # 4096
    Q = S // 4  # 2048

    with tile.TileContext(nc) as tc, ExitStack() as ctx:
        consts = ctx.enter_context(tc.tile_pool(name="consts", bufs=1))
        # 8 bufs: slots 0-7 get fresh ring buffers so no HWDGE load ever
        # waits on a tile-free (a 6-deep ring deadlocked: the Act-queue's
        # slot-7 load waited on a buffer freed only by compute that needed
        # Act to progress past that very DMA). Only the two SWDGE loads
        # (slots 8, 9) wait, on kv0/kv1 freeing at ~12-16us.
        raw_pool = ctx.enter_context(tc.tile_pool(name="raw", bufs=8))
        tree_pool = ctx.enter_context(tc.tile_pool(name="tree", bufs=2))
        head_pool = ctx.enter_context(tc.tile_pool(name="head", bufs=3))
        qhat_pool = ctx.enter_context(tc.tile_pool(name="qhat", bufs=3))
        ex_pool = ctx.enter_context(tc.tile_pool(name="ex", bufs=4))
        psum_proj = ctx.enter_context(tc.tile_pool(name="pproj", bufs=2, space="PSUM"))
        psum_rope = ctx.enter_context(tc.tile_pool(name="prope", bufs=2, space="PSUM"))
        psum_attn = ctx.enter_context(tc.tile_pool(name="pattn", bufs=4, space="PSUM"))

        # raw tiles allocated in CONSUMPTION order (ring-buffer slot k
        # pairs with slot k-6's free), DMAs issued in per-queue priority
        # order afterwards
        def head_src(slot):
            return (k_d, slot) if slot < n_kh else (q_d, slot - n_kh)

        raw_tiles = [
            raw_pool.tile([128, S], f16, tag="x", name=f"x{s}")
            for s in range(n_heads)
        ]

        # ---- scalar-queue loads, first in Act program order ----
        # (SWDGE loads measured only ~100-160 GB/s and just stole fabric
        # bandwidth from the HWDGE queues — everything stays on the two
        # HWDGE queues. Slot 8 rides scalar: its ring buffer frees with
        # kv0's projection, a PE/DVE-only chain, so the brief Act-SEQ
        # head-of-line wait cannot deadlock.)
        x1 = raw_tiles[1]
        src, idx = head_src(1)
        nc.scalar.dma_start(out=x1[:, 0:H], in_=src[idx, :, 0:H])
        nc.scalar.dma_start(out=x1[:, H:S], in_=src[idx, :, H:S])
        for slot in (3, 5, 7, 8):
            src, idx = head_src(slot)
            nc.scalar.dma_start(out=raw_tiles[slot], in_=src[idx])

        # ---- sync queue: kv0's first quarters, then the packed consts,
        # then the rest (kv0 pooling starts ~4us in; the first projection
        # matmul only needs the consts at ~9us) ----
        x0 = raw_tiles[0]
        src, idx = head_src(0)
        for qq in range(2):
            nc.sync.dma_start(
                out=x0[:, qq * Q : (qq + 1) * Q], in_=src[idx, :, qq * Q : (qq + 1) * Q]
            )
        cpack = consts.tile([128, CW], f16)
        nc.sync.dma_start(out=cpack, in_=cpack_d)
        for qq in range(2, 4):
            nc.sync.dma_start(
                out=x0[:, qq * Q : (qq + 1) * Q], in_=src[idx, :, qq * Q : (qq + 1) * Q]
            )

        def wslab(is_q, head, chunk):
            base = O_WQ if is_q else O_WK
            o = base + (head * 2 + chunk) * HID
            return cpack[:, o : o + HID]

        rot_sb = cpack[:, O_ROT : O_ROT + HID]
        ident_sb = cpack[:, O_IDENT : O_IDENT + 128]
        cos_sb = cpack[:, O_COS : O_COS + NB]
        sin_sb = cpack[:, O_SIN : O_SIN + NB]

        def bias_sb(t):
            if causal:
                return cpack[:, O_BIAS : O_BIAS + 128]
            return cpack[:, O_BIAS + t * NB : O_BIAS + (t + 1) * NB]

        shift_sb = consts.tile([128, 1], f32)
        nc.vector.memset(shift_sb, -3.0)
        khat_all = consts.tile([HID, n_kh, NB], f16)

        for slot in (2, 4, 6):
            src, idx = head_src(slot)
            nc.sync.dma_start(out=raw_tiles[slot], in_=src[idx])
        # slot 9 (the tail head) in halves so its tree starts on the
        # first half while the second is still in flight
        src, idx = head_src(9)
        nc.sync.dma_start(out=raw_tiles[9][:, 0:H], in_=src[idx, :, 0:H])
        nc.sync.dma_start(out=raw_tiles[9][:, H:S], in_=src[idx, :, H:S])

        def max_tree(slot):
            """Halving max tree over the 16 j-slabs -> tr[:, 0:NB]."""
            x = raw_tiles[slot]
            tr = tree_pool.tile([128, H], f16, tag="tr")
            if slot == 0:
                for qq in range(4):
                    o = qq * Q // 2
                    nc.vector.tensor_max(
                        tr[:, o : o + Q // 2],
                        x[:, qq * Q : qq * Q + Q // 2],
                        x[:, qq * Q + Q // 2 : (qq + 1) * Q],
                    )
                    nc.vector.tensor_max(
                        tr[:, o : o + NB], tr[:, o : o + NB], tr[:, o + NB : o + Q // 2]
                    )
                nc.vector.tensor_max(tr[:, 0:NB], tr[:, 0:NB], tr[:, 1024:1536])
                nc.vector.tensor_max(tr[:, 2048:2560], tr[:, 2048:2560], tr[:, 3072:3584])
                nc.vector.tensor_max(tr[:, 0:NB], tr[:, 0:NB], tr[:, 2048:2560])
            elif slot in (1, 9):
                for hh in range(2):
                    o = hh * H // 2
                    nc.vector.tensor_max(
                        tr[:, o : o + H // 2],
                        x[:, hh * H : hh * H + H // 2],
                        x[:, hh * H + H // 2 : (hh + 1) * H],
                    )
                    nc.vector.tensor_max(
                        tr[:, o : o + Q // 2], tr[:, o : o + Q // 2], tr[:, o + Q // 2 : o + Q]
                    )
                    nc.vector.tensor_max(
                        tr[:, o : o + NB], tr[:, o : o + NB], tr[:, o + NB : o + Q // 2]
                    )
                nc.vector.tensor_max(tr[:, 0:NB], tr[:, 0:NB], tr[:, 2048:2560])
            else:
                nc.vector.tensor_max(tr, x[:, 0:H], x[:, H:S])
                nc.vector.tensor_max(tr[:, 0 : H // 2], tr[:, 0 : H // 2], tr[:, H // 2 : H])
                nc.vector.tensor_max(tr[:, 0 : H // 4], tr[:, 0 : H // 4], tr[:, H // 4 : H // 2])
                nc.vector.tensor_max(tr[:, 0:NB], tr[:, 0:NB], tr[:, NB : 2 * NB])
            return tr

        def pool_project_rope(slot, is_q, w_head_idx, dst_ap):
            x = raw_tiles[slot]
            tr = max_tree(slot)
            mx = tr[:, 0:NB]

            ph = psum_proj.tile([HID, NB], f32, tag="proj")
            # (a stride-0 PSUM-output matmul that fuses the 16 j-slab
            # accumulations into one instruction fails the walrus ISA
            # encoding check — keep the 16-matmul chain; the redundant
            # LDWEIGHTS between them are elided by --enable-ldw-opt)
            for j in range(16):
                nc.tensor.matmul(
                    ph,
                    lhsT=wslab(is_q, w_head_idx, 0),
                    rhs=x[:, j * NB : (j + 1) * NB],
                    start=(j == 0),
                    stop=False,
                )
            nc.tensor.matmul(
                ph, lhsT=wslab(is_q, w_head_idx, 1), rhs=mx, start=False, stop=True
            )

            # h and R@h land side by side in one [128, 2*NB] tile so the
            # cos/sin multiply is ONE DVE op against the (adjacent in
            # cpack) cos|sin block
            hr = head_pool.tile([HID, 2 * NB], f16, tag="hr")
            nc.scalar.copy(hr[:, 0:NB], ph)
            rps = psum_rope.tile([HID, NB], f32, tag="rps")
            nc.tensor.matmul(rps, lhsT=rot_sb, rhs=hr[:, 0:NB], start=True, stop=True)
            nc.scalar.copy(hr[:, NB : 2 * NB], rps)
            ab = head_pool.tile([HID, 2 * NB], f16, tag="ab")
            nc.vector.tensor_mul(ab, hr, cpack[:, O_COS : O_COS + 2 * NB])
            nc.vector.tensor_add(dst_ap, ab[:, 0:NB], ab[:, NB : 2 * NB])

        store_n = [0]

        def attn_block(i, qhat):
            kv = min(i // 4, n_kh - 1)
            ex = ex_pool.tile([128, QTILES, NB], f16, tag="ex", name=f"ex{i}")
            for t in range(QTILES):
                ni = 128 * (t + 1) if causal else NB
                att = psum_attn.tile([128, NB], f32, tag="att")
                qh_t = qhat[:, t * 128 : (t + 1) * 128]
                if causal:
                    nc.tensor.matmul(
                        att[:, ni - 128 : ni], lhsT=ident_sb, rhs=bias_sb(t),
                        start=True, stop=False,
                    )
                    nc.tensor.matmul(
                        att[:, ni - 128 : ni],
                        lhsT=qh_t,
                        rhs=khat_all[:, kv, ni - 128 : ni],
                        start=False, stop=(ni == 128),
                    )
                    if ni > 128:
                        nc.tensor.matmul(
                            att[:, 0 : ni - 128],
                            lhsT=qh_t,
                            rhs=khat_all[:, kv, 0 : ni - 128],
                            start=True, stop=True,
                        )
                else:
                    nc.tensor.matmul(
                        att[:, 0:ni], lhsT=ident_sb, rhs=bias_sb(t),
                        start=True, stop=False,
                    )
                    nc.tensor.matmul(
                        att[:, 0:ni], lhsT=qh_t, rhs=khat_all[:, kv, 0:ni],
                        start=False, stop=True,
                    )

                nc.scalar.activation(
                    ex[:, t, 0:ni], att[:, 0:ni], FX.Exp, bias=shift_sb, scale=1.0
                )
            # one packed store per q-head: DRAM row t*128+p <- SBUF (p, t)
            eng = nc.sync if store_n[0] % 2 == 0 else nc.scalar
            store_n[0] += 1
            eng.dma_start(out=out_d[i], in_=ex)

        for kv in range(n_kh):
            pool_project_rope(kv, False, kv, khat_all[:, kv, :])

        qhats = [None] * n_qh
        for i in range(n_qh):
            qhat = qhat_pool.tile([HID, NB], f16, tag="qhat", name=f"qhat{i}")
            qhats[i] = qhat
            pool_project_rope(n_kh + i, True, i, qhat)
            if i >= 1:
                attn_block(i - 1, qhats[i - 1])
        attn_block(n_qh - 1, qhats[n_qh - 1])

    nc.compile()
    return nc


def _get_program(causal):
    key = (causal, QH_PER_CORE, KH_PER_CORE)
    if key not in _PROGRAMS:
        _PROGRAMS[key] = _build_program(causal)
    return _PROGRAMS[key]


def _rot_matrix():
    r = np.zeros((HID, HID), dtype=np.float16)
    for d in range(64):
        r[d, 64 + d] = -1.0
        r[64 + d, d] = 1.0
    return np.ascontiguousarray(r.T)


def _jmajor_f16(x):
    """[h, S, D] fp32 -> transposed [h, D, S] fp16 with j-major seq order
    (seq index j*NB + blk for original position blk*BS + j)."""
    h = x.shape[0]
    xt = x.reshape(h, NB, BS, D).transpose(0, 3, 2, 1)
    return np.ascontiguousarray(xt.reshape(h, D, S).astype(np.float16))


def _prep(q, k, attention_mask, cos, sin, wq, wk):
    q = np.asarray(q, dtype=np.float32)
    k = np.asarray(k, dtype=np.float32)
    mask = np.asarray(attention_mask).astype(bool)
    cos = np.asarray(cos, dtype=np.float32)
    sin = np.asarray(sin, dtype=np.float32)
    wq = np.asarray(wq, dtype=np.float32)
    wk = np.asarray(wk, dtype=np.float32)

    tril = np.tril(np.ones((NB, NB), dtype=bool))
    causal = all(np.array_equal(mask[b, 0], tril) for b in range(B))

    wq_m = wq[:, :D, :] * (ATTN_SCALE / BS)
    wq_x = wq[:, D:, :] * ATTN_SCALE
    wk_m = wk[:, :D, :] / BS
    wk_x = wk[:, D:, :]
    # [128(d), head, chunk, hid]
    wqT = np.stack([wq_m, wq_x], axis=1).transpose(2, 0, 1, 3).astype(np.float16)
    wkT = np.stack([wk_m, wk_x], axis=1).transpose(2, 0, 1, 3).astype(np.float16)

    cosT = cos.transpose(0, 2, 1).astype(np.float16)  # [B, 128, 512]
    sinT = sin.transpose(0, 2, 1).astype(np.float16)
    rotT = _rot_matrix()
    ident128 = np.eye(128, dtype=np.float16)
    if causal:
        biasB = [
            np.where(np.tril(np.ones((128, 128), dtype=bool)), 0.0, -60000.0).astype(
                np.float16
            )
        ] * B
    else:
        nb = np.where(mask[:, 0], 0.0, -60000.0).astype(np.float16)
        biasB = [
            np.concatenate([nb[b].reshape(QTILES, 128, NB)[t] for t in range(QTILES)], axis=1)
            for b in range(B)
        ]

    in_maps = []
    for c in range(N_CORES):
        b, g = c // 4, c % 4
        qs = _jmajor_f16(q[b, 8 * g : 8 * g + 8])
        ks = _jmajor_f16(k[b, 2 * g : 2 * g + 2])
        cp = np.concatenate(
            [
                wkT[:, 2 * g : 2 * g + 2].reshape(128, -1),
                wqT[:, 8 * g : 8 * g + 8].reshape(128, -1),
                rotT,
                ident128,
                cosT[b],
                sinT[b],
                biasB[b],
            ],
            axis=1,
        )
        m = {"q16": qs, "k16": ks, "cpack": np.ascontiguousarray(cp)}
        in_maps.append(m)
    return causal, in_maps


def _postprocess(results, causal):
    out = np.zeros((B, HQ, NB, NB), dtype=np.float32)
    tril = np.tril(np.ones((NB, NB), dtype=np.float32)) if causal else None
    for c in range(N_CORES):
        b, g = c // 4, c % 4
        # device layout [head, p, t, c] -> [head, row=t*128+p, col]
        ex = (
            results[c]["attn_out"]
            .transpose(0, 2, 1, 3)
            .reshape(QH_PER_CORE, NB, NB)
            .astype(np.float32)
        )
        if causal:
            # columns beyond the causal prefix hold stale device garbage
            ex = np.where(tril > 0, ex, 0.0)
        sums = ex.sum(axis=-1, keepdims=True)
        out[b, 8 * g : 8 * g + 8] = np.where(
            sums > 0, ex / np.maximum(sums, 1e-30), np.float32(1.0 / NB)
        )
    return out


def kernel(q, k, attention_mask, cos, sin, wq, wk):
    from concourse import bass_utils

    causal, in_maps = _prep(q, k, attention_mask, cos, sin, wq, wk)
    nc = _get_program(causal)
    res = bass_utils.run_bass_kernel_spmd(nc, in_maps, core_ids=list(range(N_CORES)))
    return _postprocess(res.results, causal)
